# revision 2
# baseline (speedup 1.0000x reference)
"""BitLinear (ternary-weight linear + per-row int8 fake-quant) on 8 TRN2 cores.

v3: fp8e4 DoubleRow matmuls (0.5 cyc/row) via an exact integer split of the
activations: n = hi + lo with hi = 16*round(n/16) in 16*[-8..8] and
lo in [-8..8] -- both exactly representable in fp8e4, so with ternary W the
arithmetic stays exact. Each DoubleRow matmul contracts the (hi, lo) pair
against a stride-0 broadcast of the same W slice. The stationary (hi,lo)
operand is reused across 4 consecutive matmuls (nb-inner) to amortize
LDWEIGHTS (measured 152 ns/MM at N=512 vs 224 without reuse).

Carried over from v2:
  - mean shard = local ws rows [0,1024) on every core (r=1 cores get a
    row-rotated W shard; host un-permutes output columns) -> no wm input.
  - W ternarize transposes on the DMA xbar (PE does only matmuls).
  - out stored bf16 (host casts to f32); exact-integer matmul in bf16 with
    fp32 PSUM accumulation, per-token scale in the epilogue.
  - nb-blocked main loop in m-quarters of 8 (one PSUM bank per m-tile):
    matmuls for out-block nb gate only on W tiles 4nb..4nb+3 (auto data
    deps through wT), so the PE starts right after W's first nb block is
    ternarized instead of after the full W prelude.
"""

import functools
from contextlib import ExitStack

import numpy as np

import concourse.bass as bass
import concourse.mybir as mybir
import concourse.tile as tile
from concourse.masks import make_identity
from concourse import bacc
from concourse.bass_utils import run_bass_kernel_spmd

P = 128
MAGIC = 12582912.0  # 1.5 * 2**23: forces round-to-nearest-even at integer granularity

F32 = mybir.dt.float32
BF16 = mybir.dt.bfloat16
FP8 = mybir.dt.float8e4
DRMODE = mybir.MatmulPerfMode.DoubleRow
X = mybir.AxisListType.X
ALU = mybir.AluOpType
ACTF = mybir.ActivationFunctionType


def _body(tc, xs, ws, wsc, out, *, KC, MT, NB, WT_TILES, MEAN_TILES, NB_FREE,
          MQ, n_cores, total_w_elems):
    nc = tc.nc
    in_dim = KC * P

    with ExitStack() as ctx:
        consts = ctx.enter_context(tc.tile_pool(name="consts", bufs=1))
        wres = ctx.enter_context(tc.tile_pool(name="wres", bufs=1))
        wldp = ctx.enter_context(tc.tile_pool(name="wldp", bufs=3))
        bfp = ctx.enter_context(tc.tile_pool(name="bfp", bufs=2))
        xldp = ctx.enter_context(tc.tile_pool(name="xldp", bufs=2))
        nqp = ctx.enter_context(tc.tile_pool(name="nqp", bufs=2))
        ntp = ctx.enter_context(tc.tile_pool(name="ntp", bufs=3))
        h32p = ctx.enter_context(tc.tile_pool(name="h32p", bufs=2))
        wttp = ctx.enter_context(tc.tile_pool(name="wttp", bufs=3))
        xqp = ctx.enter_context(tc.tile_pool(name="xqp", bufs=9))
        outp = ctx.enter_context(tc.tile_pool(name="outp", bufs=2))
        smalls = ctx.enter_context(tc.tile_pool(name="smalls", bufs=3))
        psum = ctx.enter_context(tc.tile_pool(name="psum", bufs=8, space="PSUM"))
        dram = ctx.enter_context(tc.tile_pool(name="dram", bufs=1, space="DRAM"))

        prev_xbar = {"w": None, "x": None}
        cc_ld_ref = [None]

        def chain_load(ld, key):
            # cluster xbar transposes: a later plain DMA load waits (issue
            # order) on the most recent xbar transpose of the same phase so
            # xbar<->copy DMA mode transitions amortize.
            if prev_xbar[key] is not None:
                tile.add_dep_helper(ld.ins, prev_xbar[key].ins, sync=False,
                                    reason="cluster xbar transposes")

        # ---------- Phase 0: weight_scale broadcast + constants (nothing here
        # depends on the collective, so it must come first in every FIFO).
        ones_p = consts.tile([P, 1], F32)
        nc.vector.memset(ones_p, 1.0)
        ones_f = consts.tile([1, P], F32)
        nc.vector.memset(ones_f, 1.0)
        negmagic = consts.tile([P, 1], F32)
        nc.vector.memset(negmagic, -MAGIC)
        neg16magic = consts.tile([P, 1], F32)
        nc.vector.memset(neg16magic, -16.0 * MAGIC)
        wsc1 = consts.tile([1, 1], F32)
        nc.gpsimd.dma_start(wsc1, wsc[:, :])
        ps_wsc = psum.tile([P, 1], F32, tag="ps", name="ps_wsc")
        nc.tensor.matmul(ps_wsc, ones_f, wsc1)
        wscb = consts.tile([P, 1], F32)
        nc.scalar.copy(wscb, ps_wsc)
        # epilogue scale base: -(weight_scale/254); W is stored as -2*W_t
        nwsc = consts.tile([P, 1], F32)
        nc.scalar.mul(nwsc, wscb, -1.0 / 254.0)

        # ---------- Phase 1: global mean(|W|). Every core reduces its local
        # ws rows [0, P*MEAN_TILES) (a distinct 1/8 of W by construction of
        # the host-side sharding), then a scalar AllReduce.
        acc = consts.tile([P, MEAN_TILES], F32)
        mean_last = [None]  # explicit serial-DMA sequencing across queues
        last_dma = [None]
        for i in range(MEAN_TILES):
            wtl = wldp.tile([P, in_dim], F32, tag="wld", name=f"wm_{i}")
            mean_last[0] = nc.gpsimd.dma_start(wtl, ws[i * P:(i + 1) * P, :])
            nc.vector.tensor_reduce(acc[:, i:i + 1], wtl, axis=X, op=ALU.add,
                                    apply_absolute_value=True)
        rowsum = consts.tile([P, 1], F32)
        nc.vector.tensor_reduce(rowsum, acc, axis=X, op=ALU.add)
        ps_sum = psum.tile([1, 1], F32, tag="ps", bufs=8, name="ps_sum")
        nc.tensor.matmul(ps_sum, ones_p, rowsum)
        allsum1 = consts.tile([1, 1], F32)
        nc.scalar.copy(allsum1, ps_sum)
        # broadcast + scale BEFORE the AllReduce: the collective then returns
        # the final [128,1] replicated global mean, so the only
        # post-collective step is a single SP-queue DMA (no FIFO head-of-line
        # blocking of DVE/ACT/Pool work on the collective).
        ps_b = psum.tile([P, 1], F32, tag="ps", name="ps_b")
        nc.tensor.matmul(ps_b, ones_f, allsum1)
        meanvb = consts.tile([P, 1], F32)
        nc.scalar.mul(meanvb, ps_b, 1.0 / total_w_elems)
        cc_in = dram.tile([P, 1], F32)
        cc_out = dram.tile([P, 1], F32, addr_space="Shared")
        meanv = consts.tile([P, 1], F32)

        # ---------- Phase 3: m-quarters of MQ tiles; quantize + transpose per
        # m-tile, then nb-blocked matmuls (one PSUM bank per m-tile).
        es_all = consts.tile([P, MT], F32)
        xq_tiles = [None] * MT

        def quantize_mtile(mt):
            xt = xldp.tile([P, in_dim], F32, tag="xld", name=f"x_{mt}")
            ld = nc.gpsimd.dma_start(xt, xs[mt * P:(mt + 1) * P, :])
            last_dma[0] = ld
            chain_load(ld, "x")
            mx = smalls.tile([P, 1], F32, tag="mx", name=f"mx_{mt}")
            nc.vector.tensor_reduce(mx, xt, axis=X, op=ALU.max,
                                    apply_absolute_value=True)
            dd = smalls.tile([P, 1], F32, tag="dd", name=f"dd_{mt}")
            nc.vector.tensor_scalar_add(dd, mx, 1e-8)
            rr = smalls.tile([P, 1], F32, tag="rr", name=f"rr_{mt}")
            nc.vector.reciprocal(rr, dd)
            ss = smalls.tile([P, 1], F32, tag="ss", name=f"ss_{mt}")
            nc.vector.tensor_scalar_mul(ss, rr, 127.0)  # s = 127/(max+1e-8)
            # epilogue scale: -(weight_scale * (max+1e-8) / 127)
            nc.vector.tensor_scalar(es_all[:, mt:mt + 1], dd, nwsc, None,
                                    ALU.mult)
            # n + MAGIC = fl(fl(x*s) + MAGIC)  (matches jax round-half-even);
            # computed in place over the x tile.
            nc.vector.tensor_scalar(xt, xt, ss, MAGIC, ALU.mult, ALU.add)
            nq = nqp.tile([P, in_dim], BF16, tag="nq", name=f"nq_{mt}")
            nc.scalar.activation(nq, xt, ACTF.Identity, bias=negmagic)
            return nq

        def mm_mtile(mt):
            xq = xq_tiles[mt]
            pss = [psum.tile([P, NB_FREE], F32, tag="ps", name=f"ps_{mt}_{nb}")
                   for nb in range(NB)]
            for kc in range(KC):
                # same stationary (hi,lo) pair for 4 consecutive matmuls
                for nb in range(NB):
                    rbc = wT[:, kc, None,
                             nb * NB_FREE:(nb + 1) * NB_FREE].to_broadcast(
                                 [P, 2, NB_FREE])
                    nc.tensor.matmul(
                        pss[nb], xq[:, kc, :, :], rbc,
                        start=(kc == 0), stop=(kc == KC - 1),
                        perf_mode=DRMODE,
                    )
            for nb in range(NB):
                ot = outp.tile([P, NB_FREE], BF16, tag="ot",
                               name=f"ot_{mt}_{nb}")
                nc.scalar.mul(ot, pss[nb], es_all[:, mt:mt + 1])
                nc.scalar.dma_start(
                    out[mt * P:(mt + 1) * P,
                        nb * NB_FREE:(nb + 1) * NB_FREE], ot)

        def transpose_mtile(mt, nq):
            nT = ntp.tile([P, KC, P], BF16, tag="nT", name=f"nT_{mt}")
            prev_xbar["x"] = nc.sync.dma_start_transpose(nT, nq)
            # exact split n = hi + lo (both fp8e4-exact):
            #   h32 = MAGIC + round(n/16);  hi = 16*h32 - 16*MAGIC
            #   lo  = n - hi  in [-8, 8]
            h32 = h32p.tile([P, KC, P], F32, tag="h32", name=f"h32_{mt}")
            nc.vector.tensor_scalar(h32, nT, 1.0 / 16.0, MAGIC, ALU.mult,
                                    ALU.add)
            xq = xqp.tile([P, KC, 2, P], FP8, tag="xq", name=f"xq_{mt}")
            nc.scalar.activation(xq[:, :, 0, :], h32, ACTF.Identity,
                                 scale=16.0, bias=neg16magic)
            nc.vector.tensor_tensor(xq[:, :, 1, :], nT, xq[:, :, 0, :],
                                    ALU.subtract)
            xq_tiles[mt] = xq

        # Pre-emit quantization for the first PRE m-tiles so DVE/ACT/DMA fill
        # the AllReduce wait and the xq stockpile feeds q0's matmuls while the
        # DMA focuses on the W pipeline.
        PRE = 8
        for g in range(0, PRE, 4):
            nqs = [(mt, quantize_mtile(mt)) for mt in range(g, g + 4)]
            for mt, nq in nqs:
                transpose_mtile(mt, nq)
        # The collective is emitted AFTER the PRE x loads: as a Pool-queue
        # instruction it head-blocks the queue for its full duration, so
        # everything the prelude needs from that queue must be enqueued first.
        cc_ld = nc.gpsimd.dma_start(cc_in, meanvb)
        tile.add_dep_helper(cc_ld.ins, mean_last[0].ins, sync=False,
                            reason="cc_in right after mean loads on the DMA")
        cc_ld_ref[0] = cc_ld
        nc.gpsimd.collective_compute(
            "AllReduce", ALU.add,
            replica_groups=[list(range(n_cores))],
            ins=[cc_in], outs=[cc_out],
        )
        # AllReduce result (the replicated global mean): loaded on SP after
        # the PRE transposes so it blocks neither the Pool load queue nor the
        # ACT nq chain (it is only ready once the collective completes).
        nc.sync.dma_start(meanv, cc_out)
        negmeanv = consts.tile([P, 1], F32)
        nc.vector.tensor_scalar_mul(negmeanv, meanv, -1.0)

        # ---------- Phase 2: ternarize W (as -2*W_t; the /2 and sign fold
        # into the epilogue scale), transpose on the xbar. Tiles alternate
        # between an ACT path and a DVE path so both engines share the work:
        #   ACT: s1 = Sign(mean - w); s2 = Sign(-mean - w); wtn = s1 + s2
        #   DVE: a2 = 2*(w > mean);   b2 = 2*(w < -mean);   wtn = b2 - a2
        wT = wres.tile([P, KC, WT_TILES * P], FP8)
        GX = 4

        def ternarize(i):
            wtl = wldp.tile([P, in_dim], F32, tag="wld", name=f"w_{i}")
            ld = nc.scalar.dma_start(wtl, ws[i * P:(i + 1) * P, :])
            if i == 0 and cc_ld_ref[0] is not None:
                # don't let W loads delay the collective input transfer
                tile.add_dep_helper(ld.ins, cc_ld_ref[0].ins, sync=False,
                                    reason="cc_in before W loads on the DMA")
            chain_load(ld, "w")
            wtn = bfp.tile([P, in_dim], BF16, tag="bc", bufs=3, name=f"wtn_{i}")
            if i % 2 == 0:
                s1 = bfp.tile([P, in_dim], BF16, tag="ba", bufs=2,
                              name=f"ws1_{i}")
                nc.scalar.activation(s1, wtl, ACTF.Sign, bias=meanv, scale=-1.0)
                s2 = bfp.tile([P, in_dim], BF16, tag="bb", bufs=2,
                              name=f"ws2_{i}")
                nc.scalar.activation(s2, wtl, ACTF.Sign, bias=negmeanv,
                                     scale=-1.0)
                nc.vector.tensor_tensor(wtn, s1, s2, ALU.add)
            else:
                a2 = bfp.tile([P, in_dim], BF16, tag="ba", bufs=2,
                              name=f"wa_{i}")
                nc.vector.tensor_scalar(a2, wtl, meanv, 2.0, ALU.is_gt,
                                        ALU.mult)
                b2 = bfp.tile([P, in_dim], BF16, tag="bb", bufs=2,
                              name=f"wb_{i}")
                nc.vector.tensor_scalar(b2, wtl, negmeanv, 2.0, ALU.is_lt,
                                        ALU.mult)
                nc.vector.tensor_tensor(wtn, b2, a2, ALU.subtract)
            return wtn

        for g in range(0, WT_TILES, GX):
            wtns = [(i, ternarize(i)) for i in range(g, g + GX)]
            wtts = []
            for i, wtn in wtns:
                wtt = wttp.tile([P, KC, P], BF16, tag="wtt", name=f"wtt_{i}")
                prev_xbar["w"] = nc.sync.dma_start_transpose(wtt, wtn)
                wtts.append((i, wtt))
            for i, wtt in wtts:
                # convert-copy into the resident fp8 W (engines alternate)
                dst = wT[:, :, i * P:(i + 1) * P]
                if i % 2 == 0:
                    nc.vector.tensor_copy(dst, wtt)
                else:
                    nc.scalar.copy(dst, wtt)

        # software-pipelined: emit quant batch g+1 before matmul batch g so
        # the engines always have the next batch queued (no batch-boundary
        # bubbles from emission order).
        def quant_batch(g):
            nqs = [(mt, quantize_mtile(mt)) for mt in range(g, g + 4)]
            for mt, nq in nqs:
                transpose_mtile(mt, nq)

        NBATCH = MT // 4
        for gi in range(NBATCH):
            nxt = (gi + 1) * 4
            if nxt >= PRE and nxt < MT:
                quant_batch(nxt)
            for mt in range(gi * 4, gi * 4 + 4):
                mm_mtile(mt)


def build_nc(*, tok_sh, in_dim, out_sh, n_cores=8, nb_free=512, mq=8):
    assert in_dim % P == 0 and tok_sh % P == 0 and out_sh % nb_free == 0
    nc = bacc.Bacc("TRN2", target_bir_lowering=False, debug=False,
                   num_devices=n_cores)
    xs = nc.dram_tensor("xs", [tok_sh, in_dim], F32, kind="ExternalInput")
    ws = nc.dram_tensor("ws", [out_sh, in_dim], F32, kind="ExternalInput")
    wsc = nc.dram_tensor("wsc", [1, 1], F32, kind="ExternalInput")
    out = nc.dram_tensor("out", [tok_sh, out_sh], BF16, kind="ExternalOutput")
    with tile.TileContext(nc) as tc:
        _body(
            tc, xs, ws, wsc, out,
            KC=in_dim // P, MT=tok_sh // P, NB=out_sh // nb_free,
            WT_TILES=out_sh // P, MEAN_TILES=out_sh // P // 2,
            NB_FREE=nb_free, MQ=mq, n_cores=n_cores,
            total_w_elems=float(out_sh * 4 * in_dim),
        )
    nc.compile()
    return nc


# ------------------------------------------------------------------ full-size
TOK = 8192          # 4*2048 tokens
IN_DIM = 2048
OUT_TOT = 8192
R, C = 2, 4         # token halves x out-feature quarters
TOK_SH = TOK // R
OUT_SH = OUT_TOT // C
MEAN_SH = OUT_SH // 2   # local ws rows [0, 1024) are this core's mean shard


@functools.lru_cache(maxsize=1)
def _full_nc():
    return build_nc(tok_sh=TOK_SH, in_dim=IN_DIM, out_sh=OUT_SH)


def make_in_maps(x, weight, weight_scale):
    x = np.ascontiguousarray(np.asarray(x, dtype=np.float32)).reshape(TOK, IN_DIM)
    w = np.ascontiguousarray(np.asarray(weight, dtype=np.float32))
    wsc = np.asarray(weight_scale, dtype=np.float32).reshape(1, 1)
    in_maps = []
    for d in range(8):
        r, c = divmod(d, C)
        wq = w[c * OUT_SH:(c + 1) * OUT_SH]
        if r == 1:
            # rotate rows by MEAN_SH so local rows [0, MEAN_SH) are the
            # second half of the quarter -> the 8 cores' mean shards are
            # disjoint and cover W. assemble() un-permutes out columns.
            wq = np.concatenate([wq[MEAN_SH:], wq[:MEAN_SH]], axis=0)
        in_maps.append({
            "xs": x[r * TOK_SH:(r + 1) * TOK_SH],
            "ws": np.ascontiguousarray(wq),
            "wsc": wsc,
        })
    return in_maps


def assemble(results):
    out = np.empty((TOK, OUT_TOT), dtype=np.float32)
    for d in range(8):
        r, c = divmod(d, C)
        o = np.asarray(results[d]["out"], dtype=np.float32)
        if r == 1:
            o = np.concatenate([o[:, MEAN_SH:], o[:, :MEAN_SH]], axis=1)
        out[r * TOK_SH:(r + 1) * TOK_SH, c * OUT_SH:(c + 1) * OUT_SH] = o
    return out.reshape(4, 2048, OUT_TOT)


def kernel(x, weight, weight_scale):
    nc = _full_nc()
    in_maps = make_in_maps(x, weight, weight_scale)
    res = run_bass_kernel_spmd(nc, in_maps, core_ids=list(range(8)))
    return assemble(res.results)


# revision 3
# speedup vs baseline: 1.3695x; 1.3695x over previous
"""BitLinear (ternary-weight linear + per-row int8 fake-quant) on 8 TRN2 cores.

v3: fp8e4 DoubleRow matmuls (0.5 cyc/row) via an exact integer split of the
activations: n = hi + lo with hi = 16*round(n/16) in 16*[-8..8] and
lo in [-8..8] -- both exactly representable in fp8e4, so with ternary W the
arithmetic stays exact. Each DoubleRow matmul contracts the (hi, lo) pair
against a stride-0 broadcast of the same W slice. The stationary (hi,lo)
operand is reused across 4 consecutive matmuls (nb-inner) to amortize
LDWEIGHTS (measured 152 ns/MM at N=512 vs 224 without reuse).

Carried over from v2:
  - mean shard = local ws rows [0,1024) on every core (r=1 cores get a
    row-rotated W shard; host un-permutes output columns) -> no wm input.
  - W ternarize transposes on the DMA xbar (PE does only matmuls).
  - out stored bf16 (host casts to f32); exact-integer matmul in bf16 with
    fp32 PSUM accumulation, per-token scale in the epilogue.
  - nb-blocked main loop in m-quarters of 8 (one PSUM bank per m-tile):
    matmuls for out-block nb gate only on W tiles 4nb..4nb+3 (auto data
    deps through wT), so the PE starts right after W's first nb block is
    ternarized instead of after the full W prelude.
"""

import functools
from contextlib import ExitStack

import numpy as np

import concourse.bass as bass
import concourse.mybir as mybir
import concourse.tile as tile
from concourse.masks import make_identity
from concourse import bacc
from concourse.bass_utils import run_bass_kernel_spmd

P = 128
MAGIC = 12582912.0  # 1.5 * 2**23: forces round-to-nearest-even at integer granularity

F32 = mybir.dt.float32
BF16 = mybir.dt.bfloat16
FP8 = mybir.dt.float8e4
DRMODE = mybir.MatmulPerfMode.DoubleRow
X = mybir.AxisListType.X
ALU = mybir.AluOpType
ACTF = mybir.ActivationFunctionType


def _body(tc, xs, ws, wsc, out, *, KC, MT, NB, WT_TILES, MEAN_TILES, NB_FREE,
          MQ, n_cores, total_w_elems):
    nc = tc.nc
    in_dim = KC * P

    with ExitStack() as ctx:
        consts = ctx.enter_context(tc.tile_pool(name="consts", bufs=1))
        wres = ctx.enter_context(tc.tile_pool(name="wres", bufs=1))
        wldp = ctx.enter_context(tc.tile_pool(name="wldp", bufs=3))
        bfp = ctx.enter_context(tc.tile_pool(name="bfp", bufs=2))
        xldp = ctx.enter_context(tc.tile_pool(name="xldp", bufs=3))
        nqp = ctx.enter_context(tc.tile_pool(name="nqp", bufs=2))
        ntp = ctx.enter_context(tc.tile_pool(name="ntp", bufs=3))
        h32p = ctx.enter_context(tc.tile_pool(name="h32p", bufs=2))
        wttp = ctx.enter_context(tc.tile_pool(name="wttp", bufs=3))
        xqp = ctx.enter_context(tc.tile_pool(name="xqp", bufs=9))
        outp = ctx.enter_context(tc.tile_pool(name="outp", bufs=2))
        smalls = ctx.enter_context(tc.tile_pool(name="smalls", bufs=3))
        psum = ctx.enter_context(tc.tile_pool(name="psum", bufs=8, space="PSUM"))
        dram = ctx.enter_context(tc.tile_pool(name="dram", bufs=1, space="DRAM"))

        prev_xbar = {"w": None, "x": None}
        cc_ld_ref = [None]

        def chain_load(ld, key):
            # cluster xbar transposes: a later plain DMA load waits (issue
            # order) on the most recent xbar transpose of the same phase so
            # xbar<->copy DMA mode transitions amortize.
            if prev_xbar[key] is not None:
                tile.add_dep_helper(ld.ins, prev_xbar[key].ins, sync=False,
                                    reason="cluster xbar transposes")

        # ---------- Phase 0: weight_scale broadcast + constants (nothing here
        # depends on the collective, so it must come first in every FIFO).
        ones_p = consts.tile([P, 1], F32)
        nc.vector.memset(ones_p, 1.0)
        ones_f = consts.tile([1, P], F32)
        nc.vector.memset(ones_f, 1.0)
        negmagic = consts.tile([P, 1], F32)
        nc.vector.memset(negmagic, -MAGIC)
        neg16magic = consts.tile([P, 1], F32)
        nc.vector.memset(neg16magic, -16.0 * MAGIC)
        wsc1 = consts.tile([1, 1], F32)
        nc.gpsimd.dma_start(wsc1, wsc[:, :])
        ps_wsc = psum.tile([P, 1], F32, tag="ps", name="ps_wsc")
        nc.tensor.matmul(ps_wsc, ones_f, wsc1)
        wscb = consts.tile([P, 1], F32)
        nc.scalar.copy(wscb, ps_wsc)
        # epilogue scale base: -(weight_scale/254); W is stored as -2*W_t
        nwsc = consts.tile([P, 1], F32)
        nc.scalar.mul(nwsc, wscb, -1.0 / 254.0)

        # ---------- Phase 1: global mean(|W|). Every core reduces its local
        # ws rows [0, P*MEAN_TILES) (a distinct 1/8 of W by construction of
        # the host-side sharding), then a scalar AllReduce.
        acc = consts.tile([P, MEAN_TILES], F32)
        mean_last = [None]  # explicit serial-DMA sequencing across queues
        last_dma = [None]
        for i in range(MEAN_TILES):
            wtl = wldp.tile([P, in_dim], F32, tag="wld", name=f"wm_{i}")
            mean_last[0] = nc.gpsimd.dma_start(wtl, ws[i * P:(i + 1) * P, :])
            nc.vector.tensor_reduce(acc[:, i:i + 1], wtl, axis=X, op=ALU.add,
                                    apply_absolute_value=True)
        rowsum = consts.tile([P, 1], F32)
        nc.vector.tensor_reduce(rowsum, acc, axis=X, op=ALU.add)
        # single matmul does sum-over-partitions AND broadcast to [128,1]:
        # out[m] = sum_k ones[k,m] * rowsum[k]; scale BEFORE the AllReduce so
        # the collective returns the final replicated global mean and the only
        # post-collective step is one SP-queue DMA.
        ones_pp = consts.tile([P, P], F32)
        nc.vector.memset(ones_pp, 1.0)
        ps_b = psum.tile([P, 1], F32, tag="ps", bufs=8, name="ps_b")
        nc.tensor.matmul(ps_b, ones_pp, rowsum)
        meanvb = consts.tile([P, 1], F32)
        nc.scalar.mul(meanvb, ps_b, 1.0 / total_w_elems)
        cc_in = dram.tile([P, 1], F32)
        cc_out = dram.tile([P, 1], F32, addr_space="Shared")
        meanv = consts.tile([P, 1], F32)

        # ---------- Phase 3: m-quarters of MQ tiles; quantize + transpose per
        # m-tile, then nb-blocked matmuls (one PSUM bank per m-tile).
        es_all = consts.tile([P, MT], F32)
        xq_tiles = [None] * MT

        def quantize_mtile(mt):
            xt = xldp.tile([P, in_dim], F32, tag="xld", name=f"x_{mt}")
            ld = nc.gpsimd.dma_start(xt, xs[mt * P:(mt + 1) * P, :])
            last_dma[0] = ld
            chain_load(ld, "x")
            mx = smalls.tile([P, 1], F32, tag="mx", name=f"mx_{mt}")
            nc.vector.tensor_reduce(mx, xt, axis=X, op=ALU.max,
                                    apply_absolute_value=True)
            dd = smalls.tile([P, 1], F32, tag="dd", name=f"dd_{mt}")
            nc.vector.tensor_scalar_add(dd, mx, 1e-8)
            rr = smalls.tile([P, 1], F32, tag="rr", name=f"rr_{mt}")
            nc.vector.reciprocal(rr, dd)
            ss = smalls.tile([P, 1], F32, tag="ss", name=f"ss_{mt}")
            nc.vector.tensor_scalar_mul(ss, rr, 127.0)  # s = 127/(max+1e-8)
            # epilogue scale: -(weight_scale * (max+1e-8) / 127)
            nc.vector.tensor_scalar(es_all[:, mt:mt + 1], dd, nwsc, None,
                                    ALU.mult)
            # n + MAGIC = fl(fl(x*s) + MAGIC)  (matches jax round-half-even);
            # computed in place over the x tile.
            nc.vector.tensor_scalar(xt, xt, ss, MAGIC, ALU.mult, ALU.add)
            nq = nqp.tile([P, in_dim], BF16, tag="nq", name=f"nq_{mt}")
            nc.scalar.activation(nq, xt, ACTF.Identity, bias=negmagic)
            return nq

        def mm_mtile(mt):
            xq = xq_tiles[mt]
            pss = [psum.tile([P, NB_FREE], F32, tag="ps", name=f"ps_{mt}_{nb}")
                   for nb in range(NB)]
            for kc in range(KC):
                # same stationary (hi,lo) pair for 4 consecutive matmuls
                for nb in range(NB):
                    rbc = wT[:, kc, None,
                             nb * NB_FREE:(nb + 1) * NB_FREE].to_broadcast(
                                 [P, 2, NB_FREE])
                    nc.tensor.matmul(
                        pss[nb], xq[:, kc, :, :], rbc,
                        start=(kc == 0), stop=(kc == KC - 1),
                        perf_mode=DRMODE,
                    )
            for nb in range(NB):
                ot = outp.tile([P, NB_FREE], BF16, tag="ot",
                               name=f"ot_{mt}_{nb}")
                nc.scalar.mul(ot, pss[nb], es_all[:, mt:mt + 1])
                nc.scalar.dma_start(
                    out[mt * P:(mt + 1) * P,
                        nb * NB_FREE:(nb + 1) * NB_FREE], ot)

        def transpose_mtile(mt, nq):
            nT = ntp.tile([P, KC, P], BF16, tag="nT", name=f"nT_{mt}")
            prev_xbar["x"] = nc.sync.dma_start_transpose(nT, nq)
            # exact split n = hi + lo (both fp8e4-exact):
            #   h32 = MAGIC + round(n/16);  hi = 16*h32 - 16*MAGIC
            #   lo  = n - hi  in [-8, 8]
            h32 = h32p.tile([P, KC, P], F32, tag="h32", name=f"h32_{mt}")
            nc.vector.tensor_scalar(h32, nT, 1.0 / 16.0, MAGIC, ALU.mult,
                                    ALU.add)
            xq = xqp.tile([P, KC, 2, P], FP8, tag="xq", name=f"xq_{mt}")
            nc.scalar.activation(xq[:, :, 0, :], h32, ACTF.Identity,
                                 scale=16.0, bias=neg16magic)
            nc.vector.tensor_tensor(xq[:, :, 1, :], nT, xq[:, :, 0, :],
                                    ALU.subtract)
            xq_tiles[mt] = xq

        # Pre-emit quantization for the first PRE m-tiles so DVE/ACT/DMA fill
        # the AllReduce wait and the xq stockpile feeds q0's matmuls while the
        # DMA focuses on the W pipeline.
        PRE = 8
        for g in range(0, PRE, 4):
            nqs = [(mt, quantize_mtile(mt)) for mt in range(g, g + 4)]
            for mt, nq in nqs:
                transpose_mtile(mt, nq)
        # The collective is emitted AFTER the PRE x loads: as a Pool-queue
        # instruction it head-blocks the queue for its full duration, so
        # everything the prelude needs from that queue must be enqueued first.
        cc_ld = nc.gpsimd.dma_start(cc_in, meanvb)
        tile.add_dep_helper(cc_ld.ins, mean_last[0].ins, sync=False,
                            reason="cc_in right after mean loads on the DMA")
        cc_ld_ref[0] = cc_ld
        nc.gpsimd.collective_compute(
            "AllReduce", ALU.add,
            replica_groups=[list(range(n_cores))],
            ins=[cc_in], outs=[cc_out],
        )
        # AllReduce result (the replicated global mean): loaded on SP after
        # the PRE transposes so it blocks neither the Pool load queue nor the
        # ACT nq chain (it is only ready once the collective completes).
        nc.sync.dma_start(meanv, cc_out)
        negmeanv = consts.tile([P, 1], F32)
        nc.vector.tensor_scalar_mul(negmeanv, meanv, -1.0)

        # ---------- Phase 2: ternarize W (as -2*W_t; the /2 and sign fold
        # into the epilogue scale), transpose on the xbar. Tiles alternate
        # between an ACT path and a DVE path so both engines share the work:
        #   ACT: s1 = Sign(mean - w); s2 = Sign(-mean - w); wtn = s1 + s2
        #   DVE: a2 = 2*(w > mean);   b2 = 2*(w < -mean);   wtn = b2 - a2
        wT = wres.tile([P, KC, WT_TILES * P], FP8)
        GX = 4

        def ternarize(i):
            wtl = wldp.tile([P, in_dim], F32, tag="wld", name=f"w_{i}")
            ld = nc.scalar.dma_start(wtl, ws[i * P:(i + 1) * P, :])
            if i == 0 and cc_ld_ref[0] is not None:
                # don't let W loads delay the collective input transfer
                tile.add_dep_helper(ld.ins, cc_ld_ref[0].ins, sync=False,
                                    reason="cc_in before W loads on the DMA")
            chain_load(ld, "w")
            wtn = bfp.tile([P, in_dim], BF16, tag="bc", bufs=3, name=f"wtn_{i}")
            if i % 2 == 0:
                s1 = bfp.tile([P, in_dim], BF16, tag="ba", bufs=2,
                              name=f"ws1_{i}")
                nc.scalar.activation(s1, wtl, ACTF.Sign, bias=meanv, scale=-1.0)
                s2 = bfp.tile([P, in_dim], BF16, tag="bb", bufs=2,
                              name=f"ws2_{i}")
                nc.scalar.activation(s2, wtl, ACTF.Sign, bias=negmeanv,
                                     scale=-1.0)
                nc.vector.tensor_tensor(wtn, s1, s2, ALU.add)
            else:
                a2 = bfp.tile([P, in_dim], BF16, tag="ba", bufs=2,
                              name=f"wa_{i}")
                nc.vector.tensor_scalar(a2, wtl, meanv, 2.0, ALU.is_gt,
                                        ALU.mult)
                b2 = bfp.tile([P, in_dim], BF16, tag="bb", bufs=2,
                              name=f"wb_{i}")
                nc.vector.tensor_scalar(b2, wtl, negmeanv, 2.0, ALU.is_lt,
                                        ALU.mult)
                nc.vector.tensor_tensor(wtn, b2, a2, ALU.subtract)
            return wtn

        for g in range(0, WT_TILES, GX):
            wtns = [(i, ternarize(i)) for i in range(g, g + GX)]
            wtts = []
            for i, wtn in wtns:
                wtt = wttp.tile([P, KC, P], BF16, tag="wtt", name=f"wtt_{i}")
                prev_xbar["w"] = nc.sync.dma_start_transpose(wtt, wtn)
                wtts.append((i, wtt))
            for i, wtt in wtts:
                # convert-copy into the resident fp8 W (engines alternate)
                dst = wT[:, :, i * P:(i + 1) * P]
                if i % 2 == 0:
                    nc.vector.tensor_copy(dst, wtt)
                else:
                    nc.scalar.copy(dst, wtt)

        # software-pipelined: emit quant batch g+1 before matmul batch g so
        # the engines always have the next batch queued (no batch-boundary
        # bubbles from emission order).
        def quant_batch(g):
            nqs = [(mt, quantize_mtile(mt)) for mt in range(g, g + 4)]
            for mt, nq in nqs:
                transpose_mtile(mt, nq)

        NBATCH = MT // 4
        for gi in range(NBATCH):
            nxt = (gi + 1) * 4
            if nxt >= PRE and nxt < MT:
                quant_batch(nxt)
            for mt in range(gi * 4, gi * 4 + 4):
                mm_mtile(mt)


def build_nc(*, tok_sh, in_dim, out_sh, n_cores=8, nb_free=512, mq=8):
    assert in_dim % P == 0 and tok_sh % P == 0 and out_sh % nb_free == 0
    nc = bacc.Bacc("TRN2", target_bir_lowering=False, debug=False,
                   num_devices=n_cores)
    xs = nc.dram_tensor("xs", [tok_sh, in_dim], F32, kind="ExternalInput")
    ws = nc.dram_tensor("ws", [out_sh, in_dim], F32, kind="ExternalInput")
    wsc = nc.dram_tensor("wsc", [1, 1], F32, kind="ExternalInput")
    out = nc.dram_tensor("out", [tok_sh, out_sh], BF16, kind="ExternalOutput")
    with tile.TileContext(nc) as tc:
        _body(
            tc, xs, ws, wsc, out,
            KC=in_dim // P, MT=tok_sh // P, NB=out_sh // nb_free,
            WT_TILES=out_sh // P, MEAN_TILES=out_sh // P // 2,
            NB_FREE=nb_free, MQ=mq, n_cores=n_cores,
            total_w_elems=float(out_sh * 4 * in_dim),
        )
    nc.compile()
    return nc


# ------------------------------------------------------------------ full-size
TOK = 8192          # 4*2048 tokens
IN_DIM = 2048
OUT_TOT = 8192
R, C = 2, 4         # token halves x out-feature quarters
TOK_SH = TOK // R
OUT_SH = OUT_TOT // C
MEAN_SH = OUT_SH // 2   # local ws rows [0, 1024) are this core's mean shard


@functools.lru_cache(maxsize=1)
def _full_nc():
    return build_nc(tok_sh=TOK_SH, in_dim=IN_DIM, out_sh=OUT_SH)


def make_in_maps(x, weight, weight_scale):
    x = np.ascontiguousarray(np.asarray(x, dtype=np.float32)).reshape(TOK, IN_DIM)
    w = np.ascontiguousarray(np.asarray(weight, dtype=np.float32))
    wsc = np.asarray(weight_scale, dtype=np.float32).reshape(1, 1)
    in_maps = []
    for d in range(8):
        r, c = divmod(d, C)
        wq = w[c * OUT_SH:(c + 1) * OUT_SH]
        if r == 1:
            # rotate rows by MEAN_SH so local rows [0, MEAN_SH) are the
            # second half of the quarter -> the 8 cores' mean shards are
            # disjoint and cover W. assemble() un-permutes out columns.
            wq = np.concatenate([wq[MEAN_SH:], wq[:MEAN_SH]], axis=0)
        in_maps.append({
            "xs": x[r * TOK_SH:(r + 1) * TOK_SH],
            "ws": np.ascontiguousarray(wq),
            "wsc": wsc,
        })
    return in_maps


def assemble(results):
    out = np.empty((TOK, OUT_TOT), dtype=np.float32)
    for d in range(8):
        r, c = divmod(d, C)
        o = np.asarray(results[d]["out"], dtype=np.float32)
        if r == 1:
            o = np.concatenate([o[:, MEAN_SH:], o[:, :MEAN_SH]], axis=1)
        out[r * TOK_SH:(r + 1) * TOK_SH, c * OUT_SH:(c + 1) * OUT_SH] = o
    return out.reshape(4, 2048, OUT_TOT)


def kernel(x, weight, weight_scale):
    nc = _full_nc()
    in_maps = make_in_maps(x, weight, weight_scale)
    res = run_bass_kernel_spmd(nc, in_maps, core_ids=list(range(8)))
    return assemble(res.results)
